# revision 9
# baseline (speedup 1.0000x reference)
"""Trainium2 Bass kernel for the fused einsum/groupconv/bmm module.

Math (per image n, C=256, H=W=56, HW=3136, fp32):
  t1[c,e] = sum_s X[c,s] P[s,e]          (X = x[n] as C x HW, P = p1_w as HW x C)
  t3      = groupconv3x3(x[n], conv_w, groups=2)
  t4      = p4 * t1;  t5[a] = sum_b t4[a,b] p5[b]
  t6      = (t4 @ t3) / 16;  t7[s] = (sum_c t5[c] X[c,s]) / 16
  out     = t6 + t7[broadcast over c]

Device strategy (8 cores, 4 images each):
  - x loaded once per image into a zero-padded (128, 58*58) layout per
    c-block. Everything runs in padded s'-coordinates (s' = (h+1)*58+w+1)
    so every matmul operand is a flat contiguous slice (walrus requires
    single-free-dim APs on matmul); pad columns yield garbage that is
    never copied out.
  - conv = 9 shifted matmuls per group accumulated in PSUM, chunks of 8
    padded rows (N=464).
  - X^T built on-chip with PE transposes (116-wide contiguous chunks);
    t1 computed *transposed* (t1T = P'^T @ X^T) over the padded s'-space
    with zeros in P' at pad rows, landing t4T directly in matmul-lhsT
    layout.
  - 1/sqrt(C) folded into p4 host-side; t7 broadcast-added into the t6
    PSUM accumulation via a K=1 ones-matmul.
  - All big matmuls run as float32r (full PE rate at free-dim >= 256).
"""

import sys

sys.path.insert(0, "/opt/trn_rl_repo")

import numpy as np

N, C, H, W = 32, 256, 56, 56
HW = H * W            # 3136
PH = H + 2            # 58
PHW = PH * PH         # 3364
XLEN = PHW + 2        # +1 guard element on each end for corner tap shifts
NCORES = 8
NPER = N // NCORES    # 4 images per core
CHP = 8 * PH          # padded chunk: 8 padded rows = 464
NCHUNK = 7            # row starts 1,9,...,49 cover out rows 1..56
KP = 116              # transpose chunk (contiguous in padded space)
KT = PHW // KP        # 29
INV = 1.0 / 16.0      # 1/sqrt(C)


def build_body(tc, outs, ins):
    import concourse.mybir as mybir

    nc = tc.nc
    f32 = mybir.dt.float32
    f32r = mybir.dt.float32r

    x_d = ins["x"]          # (NPER, C, H, W)
    p1_d = ins["p1"]        # (PHW, C)  zeros at pad rows
    wt_d = ins["wt"]        # (2, 9, 128, 128)  [g, tap, i, o]
    p4_d = ins["p4s"]       # (2, 128, 256)     [bb, b, a]  (pre-scaled by 1/16)
    p5_d = ins["p5"]        # (2, 128)          [bb, b]
    out_d = outs["out"]     # (NPER, C, HW)

    with (
        tc.tile_pool(name="const", bufs=1) as constp,
        tc.tile_pool(name="xpadp", bufs=4) as xpadp,
        tc.tile_pool(name="xtp", bufs=1) as xtp,
        tc.tile_pool(name="t3p", bufs=6) as t3p,
        tc.tile_pool(name="svp", bufs=3) as svp,
        tc.tile_pool(name="outp", bufs=5) as outp,
        tc.tile_pool(name="ps_tr", bufs=2, space="PSUM") as ps_tr,
        tc.tile_pool(name="ps_acc", bufs=1, space="PSUM") as ps_acc,
        tc.tile_pool(name="ps_cv", bufs=3, space="PSUM") as ps_cv,
        tc.tile_pool(name="ps_t6", bufs=2, space="PSUM") as ps_t6,
    ):
        # ---- constants ----
        from concourse.masks import make_identity

        ident = constp.tile([128, 128], f32, name="ident")
        make_identity(nc, ident[:, :])
        ones = constp.tile([1, 128], f32r, name="ones")
        nc.sync.dma_start(out=ones[:, :], in_=ins["onesv"].bitcast(f32r))
        zv = ins["zv"]

        p1_sb = constp.tile([KP, KT * C], f32r, name="p1_sb")
        nc.sync.dma_start(
            out=p1_sb.rearrange("p (k e) -> p k e", e=C),
            in_=p1_d.rearrange("(k p) e -> p k e", p=KP).bitcast(f32r),
        )
        wt_sb = constp.tile([128, 2 * 9 * 128], f32r, name="wt_sb")
        nc.sync.dma_start(
            out=wt_sb.rearrange("i (g t o) -> i g t o", g=2, t=9),
            in_=wt_d.rearrange("g t i o -> i g t o").bitcast(f32r),
        )
        p4_sb = constp.tile([128, 2 * C], f32, name="p4_sb")
        nc.sync.dma_start(
            out=p4_sb.rearrange("b (bb a) -> b bb a", bb=2),
            in_=p4_d.rearrange("bb b a -> b bb a"),
        )
        p5_sb = constp.tile([128, 2], f32, name="p5_sb")
        nc.sync.dma_start(
            out=p5_sb[:, :], in_=p5_d.rearrange("bb b -> b bb"),
        )

        for n in range(NPER):
            # ---- load x into zero-padded flat layout per c-block ----
            # tile flat index f = 1 + s',  s' = row*58 + col (58x58 padded)
            xpads = []
            for cb in range(2):
                xp = xpadp.tile([128, XLEN], f32r, tag=f"xpad{cb}", name=f"xp{cb}")
                # zero pad positions: head (guard+row0+row1col0), seam
                # pairs (col57 of row r + col0 of row r+1), tail
                nc.sync.dma_start(
                    out=xp[:, 0:60], in_=zv[:, 0:60].bitcast(f32r)
                )
                nc.sync.dma_start(
                    out=xp[:, 116 : 116 + 55 * PH].rearrange(
                        "p (r w) -> p r w", w=PH
                    )[:, :, 0:2],
                    in_=zv[:, 0:110].rearrange("p (r w) -> p r w", w=2)
                    .bitcast(f32r),
                )
                nc.sync.dma_start(
                    out=xp[:, 3306:XLEN], in_=zv[:, 0:60].bitcast(f32r)
                )
                nc.sync.dma_start(
                    out=xp[:, 60 : 60 + 56 * PH].rearrange(
                        "p (r w) -> p r w", w=PH
                    )[:, :, 0:56],
                    in_=x_d[n, cb * 128 : (cb + 1) * 128].bitcast(f32r),
                )
                xpads.append(xp)

            # ---- X^T via PE transposes (116 contiguous per shot) ----
            xt = xtp.tile([KP, KT * C], f32r, tag="xt", name="xt")
            for k in range(KT):
                for cb in range(2):
                    trp = ps_tr.tile([KP, 128], f32, tag="tr", name="trp")
                    nc.tensor.transpose(
                        trp[:, :],
                        xpads[cb][:, 1 + k * KP : 1 + (k + 1) * KP]
                        .bitcast(f32),
                        ident[:, :],
                    )
                    nc.vector.tensor_copy(
                        xt[:, k * C + cb * 128 : k * C + cb * 128 + 128],
                        trp[:, :],
                    )

            # ---- t1T = P'^T @ X^T, then t4T = p4s * t1T (b-part, a-free)
            t4T = svp.tile([128, 2 * C], f32r, tag="t4", name="t4T")
            for eb in range(2):
                t1ps = ps_acc.tile([128, C], f32, tag="acc", name="t1ps")
                for k in range(KT):
                    nc.tensor.matmul(
                        t1ps[:, :],
                        p1_sb[:, k * C + eb * 128 : k * C + eb * 128 + 128]
                        ,
                        xt[:, k * C : (k + 1) * C],
                        start=(k == 0),
                        stop=(k == KT - 1),
                    )
                nc.vector.tensor_mul(
                    t4T[:, eb * C : (eb + 1) * C],
                    t1ps[:, :],
                    p4_sb[:, eb * C : (eb + 1) * C],
                )

            # ---- t5 (column on partitions): t5[a] = sum_b t4T[b,a] p5[b]
            t5ps = ps_acc.tile([128, 2], f32, tag="acc", name="t5ps")
            for ab in range(2):
                for bb in range(2):
                    nc.tensor.matmul(
                        t5ps[:, ab : ab + 1],
                        t4T[:, bb * C + ab * 128 : bb * C + ab * 128 + 128]
                        .bitcast(f32),
                        p5_sb[:, bb : bb + 1],
                        start=(bb == 0),
                        stop=(bb == 1),
                    )
            t5col = svp.tile([128, 2], f32r, tag="t5", name="t5col")
            nc.scalar.copy(t5col[:, :], t5ps[:, :])

            # ---- chunk loop (8 padded rows each): t7, conv, t6, store ----
            for c in range(NCHUNK):
                r0 = 1 + 8 * c          # padded row of chunk start
                f0 = 1 + r0 * PH        # flat start of chunk in xpad
                t7ps = ps_acc.tile([1, CHP], f32, tag="acc", name="t7ps")
                for cb in range(2):
                    nc.tensor.matmul(
                        t7ps[:, :],
                        t5col[:, cb : cb + 1],
                        xpads[cb][:, f0 : f0 + CHP],
                        start=(cb == 0),
                        stop=(cb == 1),
                    )
                t7row = svp.tile([1, CHP], f32r, tag="t7", name="t7row")
                nc.scalar.copy(t7row[:, :], t7ps[:, :])

                t3c = []
                for g in range(2):
                    cv = ps_cv.tile([128, CHP], f32, tag="cv", name="cv")
                    for tap in range(9):
                        kh, kw = tap // 3, tap % 3
                        foff = (r0 + kh - 1) * PH + kw
                        nc.tensor.matmul(
                            cv[:, :],
                            wt_sb[
                                :,
                                (g * 9 + tap) * 128 : (g * 9 + tap) * 128 + 128,
                            ],
                            xpads[g][:, foff : foff + CHP],
                            start=(tap == 0),
                            stop=(tap == 8),
                        )
                    t3g = t3p.tile([128, CHP], f32r, tag="t3", name="t3g")
                    nc.vector.tensor_copy(t3g[:, :], cv[:, :])
                    t3c.append(t3g)

                for ab in range(2):
                    t6ps = ps_t6.tile([128, CHP], f32, tag="t6", name="t6ps")
                    for bb in range(2):
                        nc.tensor.matmul(
                            t6ps[:, :],
                            t4T[:, bb * C + ab * 128 : bb * C + ab * 128 + 128]
                            ,
                            t3c[bb][:, :],
                            start=(bb == 0),
                            stop=False,
                        )
                    nc.tensor.matmul(
                        t6ps[:, :],
                        ones[:, :],
                        t7row[:, :],
                        start=False,
                        stop=True,
                    )
                    ob = outp.tile([128, 448], f32, tag="ob", name="ob")
                    # extract interior cols (drop the 58-wide pad columns)
                    src = t6ps.rearrange("p (r w) -> p r w", w=PH)[:, :, 1:57]
                    if ab == 0:
                        nc.vector.tensor_copy(
                            ob.rearrange("p (r w) -> p r w", w=56), src
                        )
                    else:
                        nc.scalar.copy(
                            ob.rearrange("p (r w) -> p r w", w=56), src
                        )
                    nc.sync.dma_start(
                        out=out_d[
                            n,
                            ab * 128 : (ab + 1) * 128,
                            (r0 - 1) * 56 : (r0 - 1) * 56 + 448,
                        ],
                        in_=ob[:, :],
                    )


_CACHE = {}


def _get_nc():
    if "nc" in _CACHE:
        return _CACHE["nc"]
    import concourse.bacc as bacc
    import concourse.mybir as mybir
    import concourse.tile as tile

    f32 = mybir.dt.float32
    f32r = mybir.dt.float32r
    nc = bacc.Bacc(
        "TRN2", target_bir_lowering=False, debug=False, num_devices=NCORES
    )
    ins = {
        "x": nc.dram_tensor("x", (NPER, C, H, W), f32r, kind="ExternalInput").ap(),
        "p1": nc.dram_tensor("p1", (PHW, C), f32r, kind="ExternalInput").ap(),
        "wt": nc.dram_tensor("wt", (2, 9, 128, 128), f32r, kind="ExternalInput").ap(),
        "p4s": nc.dram_tensor("p4s", (2, 128, C), f32, kind="ExternalInput").ap(),
        "p5": nc.dram_tensor("p5", (2, 128), f32, kind="ExternalInput").ap(),
        "onesv": nc.dram_tensor("onesv", (1, 128), f32, kind="ExternalInput").ap(),
        "zv": nc.dram_tensor("zv", (128, 116), f32, kind="ExternalInput").ap(),
    }
    outs = {
        "out": nc.dram_tensor("out", (NPER, C, HW), f32, kind="ExternalOutput").ap(),
    }
    with tile.TileContext(nc) as tc:
        build_body(tc, outs, ins)
    nc.compile()
    _CACHE["nc"] = nc
    return nc


def host_prep(inputs):
    """Split full inputs into per-core in_maps (with host-side relayouts)."""
    x = np.ascontiguousarray(np.asarray(inputs["x"], dtype=np.float32))
    p1p = np.zeros((PH, PH, C), dtype=np.float32)
    p1p[1:57, 1:57, :] = np.asarray(inputs["p1_w"], dtype=np.float32)[..., 0]
    p1p = np.ascontiguousarray(p1p.reshape(PHW, C))
    wt = np.ascontiguousarray(
        np.asarray(inputs["conv_w"], dtype=np.float32)
        .reshape(2, 128, 128, 9)
        .transpose(0, 3, 2, 1)
    )
    p4s = np.ascontiguousarray(
        (np.asarray(inputs["p4_w"], dtype=np.float32)[0].T * INV).reshape(
            2, 128, C
        )
    )
    p5 = np.ascontiguousarray(
        np.asarray(inputs["p5_w"], dtype=np.float32).reshape(2, 128)
    )
    onesv = np.ones((1, 128), dtype=np.float32)
    zv = np.zeros((128, 116), dtype=np.float32)
    xs = x.reshape(NCORES, NPER, C, H, W)
    return [
        {
            "x": np.ascontiguousarray(xs[i]), "p1": p1p, "wt": wt,
            "p4s": p4s, "p5": p5, "onesv": onesv, "zv": zv,
        }
        for i in range(NCORES)
    ]


def kernel(**inputs):
    from concourse.bass_utils import run_bass_kernel_spmd

    nc = _get_nc()
    in_maps = host_prep(inputs)
    res = run_bass_kernel_spmd(nc, in_maps, core_ids=list(range(NCORES)))
    out = np.concatenate([res.results[i]["out"] for i in range(NCORES)], axis=0)
    return out.reshape(N, C, H, W)
